# revision 5
# baseline (speedup 1.0000x reference)
"""Reverse-time forget-mult recurrence on 8 Trainium2 NeuronCores.

h_t = f_t*x_t + (1-f_t)*h_{t+1}, h_{T+1}=0, over [T=2048, B=16, D=1024].

Memory-bound problem, so bytes/element are minimized by quantization: f -> u8
(step 1/255), x -> i8 with one global scale sx = max|x|/127, and the output
h -> i8 as well.  The recurrence is a convex combination (|1-f|<1) so
quantization error does not amplify; the device scans in the integer-scaled
domain H = (255/sx)*h, emits H * (-1/255) rounded to i8 (|H| <= 255*127 by
convexity, so the i8 range is exactly covered), and the host rescales by -sx.
Max rel err ~8e-3 vs the 2e-2 gate, validated in fp64 simulation.

The whole per-element pipeline (dequant, gate product, the sequential scan,
and output scaling) runs in ONE hand-authored custom DVE instruction at
1 element/cycle.  The stock tensor_tensor_scan costs 2 cycles/element: its
state feedback routes backward one pipeline stage (block k+1's a-flop read by
block k), which needs a one-cycle bubble between consecutive elements of one
chain.  Instead, the host interleaves TWO independent (d,b)-lanes per stream:
element e reads the a-flop written one cycle earlier, which belongs to chain
e-2 — its own chain — so the pipeline runs bubble-free at full rate.
Datapath (8 ALU blocks, elements stream through at 1/cycle, c = -1/255):

    B0: t = f * c                 B3: m = a * H_fb   (a-flop of B4)
    B1: a = 1 + t                 B4: H = m + g      (writes a-flop)
    B2: g = f * x                 B5: out = H * c    (c rides delay lane 2)
                                  B6-B7: pass-through -> i8 write

A 2-element seed uOp starts each instruction (H = g, i.e. zero initial state
for both chains), then the steady uOp runs to end-of-stream.

Layout: D is sharded across the 8 cores (128 channels -> SBUF partitions).
Each core's [128, 16, 2048] shard is laid out T-reversed and pair-interleaved
as [128, 8, 4096]: pair p holds batch rows (2p, 2p+1) alternating per element.
f loads issue on the Sync HWDGE ring; early x loads go on the Scalar ring
(idle until the first stores) so the first scan starts sooner.  One custom-DVE
instruction per pair-block [128, 4096]; i8 stores on the Scalar ring, with the
first two deferred to the kernel tail on the then-idle Sync ring.  Per-core
HBM traffic: 4+4 MB in, 4 MB out.
"""

import numpy as np

T, B, D = 2048, 16, 1024
NCORES = 8
DS = D // NCORES          # 128 channels per core -> the SBUF partition dim
PB = 128
NPAIR = B // 2            # 8 interleaved pair-blocks of [128, 2*T] per core
T2 = 2 * T
CHUNKS = (1, 1, 2, 2, 2)  # load-DMA granularity in pair-blocks
X_ON_SCALAR = 2           # chunks whose x-load rides the Scalar ring
F_SCALE = 255.0

_cached = {}


def _register_forget_scan():
    """Register the hand-authored 2-chain interleaved linear-scan DVE op."""
    import concourse.dve_ops as dve_ops
    from concourse.dve_spec import Spec, Src0, Src1
    from concourse.dve_uop import (
        ENABLE,
        AluInp,
        AluOp,
        DelayInp,
        InpSel,
        OutPath,
        OutSel,
        Trigger,
        UopConfig,
        UopDpConfig,
        DveOpSpec,
    )

    NAME = "FORGET_SCAN2S_ANT"
    if NAME in dve_ops._SUB_OPCODE_FOR_NAME:
        return dve_ops.CUSTOM_DVE_SPECS[NAME + "_op"]

    ROW = 17  # rows 1..16 taken by production OPS; byte-36 field holds < 0x20
    assert ROW not in dve_ops._SUB_OPCODE_FOR_NAME.values()

    def _reference(in0, in1, s0, s1, imm2):
        # 2-interleaved chains: H[e] = f[e]*x[e] + (1 + s0*f[e]) * H[e-2];
        # out[e] = H[e] * s0
        f = np.asarray(in0, np.float32)
        x = np.asarray(in1, np.float32)
        a = 1.0 + f * np.float32(s0)
        g = f * x
        H = np.empty_like(g)
        H[:, 0], H[:, 1] = g[:, 0], g[:, 1]
        for e in range(2, g.shape[1]):
            H[:, e] = g[:, e] + a[:, e] * H[:, e - 2]
        return H * np.float32(s0)

    def _base_uop():
        u = UopConfig()
        # delay lanes: 0=f(SRC_0) 1=x(SRC_1) 2=CONST_0 3=ONE 4=a 5=g
        u.enable_input(InpSel.SRC_0, 1)
        u.enable_input(InpSel.SRC_1, 2)
        u.enable_input(InpSel.CONST_0, 3)
        u.enable_input(InpSel.ONE_F32, 4)
        u.require_inp0 = ENABLE
        u.require_inp1 = ENABLE
        dp = u.datapath_config
        dp[0].enable_alu(AluOp.MULTIPLY, AluInp.PREV_DELAY_0, AluInp.PREV_DELAY_2)
        dp[0].pass_through_delay(0, 1, 2, 3)
        dp[1].enable_alu(AluOp.ADD, AluInp.PREV_DELAY_3, AluInp.PREV_ALU_OUT)
        dp[1].pass_through_delay(0, 1, 2)
        dp[2].enable_alu(AluOp.MULTIPLY, AluInp.PREV_DELAY_0, AluInp.PREV_DELAY_1)
        dp[2].enable_delay_from_src(DelayInp.PREV_ALU_OUT, 4)  # a -> lane4
        dp[2].pass_through_delay(2)
        dp[3].enable_alu(AluOp.MULTIPLY, AluInp.PREV_DELAY_4, AluInp.NEXT_ALU_OUT_A)
        dp[3].enable_delay_from_src(DelayInp.PREV_ALU_OUT, 5)  # g -> lane5
        dp[3].pass_through_delay(2)
        dp[4].enable_alu(AluOp.ADD, AluInp.PREV_ALU_OUT, AluInp.PREV_DELAY_5)
        dp[4].alu_out_a_enable = ENABLE  # H feedback, read by B3 next cycle
        dp[4].pass_through_delay(2)
        dp[5].enable_alu(AluOp.MULTIPLY, AluInp.PREV_ALU_OUT, AluInp.PREV_DELAY_2)
        for k in (6, 7):
            dp[k].pass_through_alu()
        u.enable_output(OutSel.ALU_OUT, OutPath.WR0_LO)
        return u

    def _build_uops(ver):
        assert ver == "v3", f"{NAME} authored for TRN2/v3 only ({ver})"
        seed = _base_uop()
        dp = seed.datapath_config
        dp[3] = UopDpConfig()
        dp[3].enable_alu(AluOp.BYPASS, AluInp.PREV_DELAY_4)
        dp[3].enable_delay_from_src(DelayInp.PREV_ALU_OUT, 5)
        dp[3].pass_through_delay(2)
        dp[4] = UopDpConfig()
        dp[4].enable_alu(AluOp.BYPASS, AluInp.PREV_DELAY_5)  # H = g (state 0)
        dp[4].alu_out_a_enable = ENABLE
        dp[4].pass_through_delay(2)
        seed.repeat_count = 2
        seed.trigger = (Trigger.SRC_TENSOR_DONE, Trigger.COUNT, Trigger.NONE)
        seed.next_uop = (0, 1, 0)
        steady = _base_uop()
        steady.trigger = (Trigger.SRC_TENSOR_DONE, Trigger.NONE, Trigger.NONE)
        steady.next_uop = (0, 0, 0)
        return [seed, steady]

    class _HandOp:
        """Duck-typed DveOp whose uOp program is hand-authored, not lowered."""

        name = NAME
        subdim = False
        spec = Spec(body=Src0 * Src1, reference=_reference)

        def __init__(self):
            self._cache = {}

        def compile(self, ver):
            if ver not in self._cache:
                s = DveOpSpec(
                    name=self.name, opcode=ROW, uops=_build_uops(ver), rd1_en=True
                )
                s.validate(ver)
                self._cache[ver] = s
            return self._cache[ver]

    op = _HandOp()
    dve_ops.OPS.append(op)
    dve_ops._SUB_OPCODE_FOR_NAME[NAME] = ROW
    dve_ops.CUSTOM_DVE_SPECS[NAME] = op.spec
    dve_ops.CUSTOM_DVE_SPECS[NAME + "_op"] = op  # stash for idempotent lookup
    return op


def _build():
    import concourse.bacc as bacc
    import concourse.mybir as mybir
    import concourse.tile as tile

    scan_op = _register_forget_scan()

    u8 = mybir.dt.uint8
    i8 = mybir.dt.int8
    nc = bacc.Bacc("TRN2", target_bir_lowering=False, debug=False, num_devices=NCORES)
    f_in = nc.dram_tensor("f_in", [PB, NPAIR, T2], u8, kind="ExternalInput").ap()
    x_in = nc.dram_tensor("x_in", [PB, NPAIR, T2], i8, kind="ExternalInput").ap()
    h_out = nc.dram_tensor("h_out", [PB, NPAIR, T2], i8, kind="ExternalOutput").ap()

    with tile.TileContext(nc) as tc:
        with (
            tc.tile_pool(name="io", bufs=3) as io_pool,
            tc.tile_pool(name="hp", bufs=4) as h_pool,
            tc.tile_pool(name="hd", bufs=1) as hd_pool,
        ):
            deferred = {}
            blk0 = 0
            for ci, cb in enumerate(CHUNKS):
                bsl = slice(blk0, blk0 + cb)
                f_t = io_pool.tile([PB, cb, T2], u8, tag="f")
                nc.sync.dma_start(out=f_t[:], in_=f_in[:, bsl, :])
                x_t = io_pool.tile([PB, cb, T2], i8, tag="x")
                # early x loads ride the Scalar ring, idle until first stores
                x_eng = nc.scalar if ci < X_ON_SCALAR else nc.sync
                x_eng.dma_start(out=x_t[:], in_=x_in[:, bsl, :])
                if ci == len(CHUNKS) - 1:
                    # Sync ring is idle after the final load: flush deferred
                    # early stores there to fill the end-of-stream DMA gap
                    for dblk, dh in deferred.items():
                        nc.sync.dma_start(out=h_out[:, dblk, :], in_=dh[:])
                for j in range(cb):
                    blk = blk0 + j
                    if blk <= 1:
                        h_t = hd_pool.tile([PB, T2], i8, tag=f"hd{blk}", name=f"hd{blk}")
                    else:
                        h_t = h_pool.tile([PB, T2], i8, tag="h")
                    nc.vector._custom_dve(
                        scan_op,
                        out=h_t[:],
                        in0=f_t[:, j, :],
                        in1=x_t[:, j, :],
                        s0=-1.0 / F_SCALE,
                    )
                    if blk <= 1:
                        deferred[blk] = h_t
                    else:
                        nc.scalar.dma_start(out=h_out[:, blk, :], in_=h_t[:])
                blk0 += cb
    nc.compile()
    return nc


def _get_nc():
    if "nc" not in _cached:
        _cached["nc"] = _build()
    return _cached["nc"]


def _shard(arr):
    """[T, B, D] -> per-core [DS, NPAIR, 2T]: T reversed, partition-major,
    batch rows (2p, 2p+1) interleaved elementwise along the stream."""
    v = arr[::-1].transpose(2, 1, 0)  # [D, B, T] strided view, T reversed
    out = []
    for c in range(NCORES):
        s = v[DS * c : DS * (c + 1)]                  # [128, 16, 2048]
        s = s.reshape(DS, NPAIR, 2, T)                # [128, 8, 2, 2048]
        out.append(np.ascontiguousarray(s.transpose(0, 1, 3, 2)).reshape(DS, NPAIR, T2))
    return out


def _run(f, x, trace=False):
    from concourse.bass_utils import run_bass_kernel_spmd

    f = np.asarray(f, dtype=np.float32)
    x = np.asarray(x, dtype=np.float32)
    assert f.shape == (T, B, D) and x.shape == (T, B, D)

    # Quantize: f -> u8 (step 1/255), x -> i8 with global scale sx.
    fq = np.rint(f * np.float32(F_SCALE)).astype(np.uint8)
    sx = float(np.abs(x).max()) / 127.0
    sx = max(sx, 1e-30)
    xq = np.clip(np.rint(x * np.float32(1.0 / sx)), -127, 127).astype(np.int8)

    nc = _get_nc()
    f_shards = _shard(fq)
    x_shards = _shard(xq)
    in_maps = [{"f_in": f_shards[c], "x_in": x_shards[c]} for c in range(NCORES)]
    res = run_bass_kernel_spmd(nc, in_maps, core_ids=list(range(NCORES)), trace=trace)

    out = np.empty((T, B, D), dtype=np.float32)
    for c in range(NCORES):
        # device emits -H/255 as i8; H_c[d, p, 2k+j] -> out[t, 2p+j, DS*c+d]
        rr = res.results[c]["h_out"].reshape(DS, NPAIR, T, 2).transpose(2, 1, 3, 0)
        out[:, :, DS * c : DS * (c + 1)] = rr.reshape(T, B, DS)[::-1]
    out *= np.float32(-sx)  # undo the device's -1/255 and the x scale
    return out.reshape(T * B, D), res


def kernel(f, x):
    return _run(f, x, trace=False)[0]


# revision 6
# speedup vs baseline: 1.1070x; 1.1070x over previous
"""Reverse-time forget-mult recurrence on 8 Trainium2 NeuronCores.

h_t = f_t*x_t + (1-f_t)*h_{t+1}, h_{T+1}=0, over [T=2048, B=16, D=1024].

Memory-bound problem, so bytes/element are minimized by quantization: f -> u8
(step 1/255), x -> i8 with one global scale sx = max|x|/127.  The recurrence
is a convex combination (|1-f|<1) so quantization error does not amplify; the
device scans in the integer-scaled domain H = (255/sx)*h with fp32 state and
fp16 output, and the host rescales by sx/255 (max rel err ~7e-3 vs the 2e-2
gate, validated in fp64 simulation).  An i8-output variant was measured
slower: the 1-byte write path throttles the scan below 1 element/cycle, which
costs more than the halved store traffic saves.

The whole per-element pipeline (dequant, gate product, and the sequential
scan) runs in ONE hand-authored custom DVE instruction at 1 element/cycle.
The stock tensor_tensor_scan costs 2 cycles/element: its state feedback
routes backward one pipeline stage (block k+1's a-flop read by block k),
which needs a one-cycle bubble between consecutive elements of one chain.
Instead, the host interleaves TWO independent (d,b)-lanes per stream:
element e reads the a-flop written one cycle earlier, which then belongs to
chain e-2 — its own chain — so the pipeline runs bubble-free at full rate.
Datapath (8 ALU blocks, elements stream through at 1/cycle, c = -1/255):

    B0: t = f * c                 B3: m = a * H_fb   (a-flop of B4)
    B1: a = 1 + t                 B4: H = m + g      (writes a-flop)
    B2: g = f * x                 B5-B7: pass-through -> fp16 write

A 2-element seed uOp starts each instruction (H = g, i.e. zero initial state
for both chains), then the steady uOp runs to end-of-stream.

Layout: D is sharded across the 8 cores (128 channels -> SBUF partitions).
Each core's [128, 16, 2048] shard is laid out T-reversed and pair-interleaved
as [128, 8, 4096]: pair p holds batch rows (2p, 2p+1) alternating per element.
f loads issue on the Sync HWDGE ring; early x loads ride the Scalar ring
(idle until the first stores).  One custom-DVE instruction per pair-block
[128, 4096]; fp16 stores on the Scalar ring, with the first two deferred to
the kernel tail on the then-idle Sync ring and the last one split across both
rings to shorten the drain.  Per-core HBM traffic: 4+4 MB in, 8 MB out.
Measured on TRN2: the 8 scans run back-to-back on Vector at 4.35 us each
(1.06 ns/element); ~7 us fixed NEFF/Tile preamble + ~2 us first-load latency
in front, ~4 us store drain + epilogue behind.
"""

import numpy as np

T, B, D = 2048, 16, 1024
NCORES = 8
DS = D // NCORES          # 128 channels per core -> the SBUF partition dim
PB = 128
NPAIR = B // 2            # 8 interleaved pair-blocks of [128, 2*T] per core
T2 = 2 * T
CHUNKS = (1, 1, 2, 2, 2)  # load-DMA granularity in pair-blocks
X_ON_SCALAR = 2           # chunks whose x-load rides the Scalar ring
F_SCALE = 255.0

_cached = {}


def _register_forget_scan():
    """Register the hand-authored 2-chain interleaved linear-scan DVE op."""
    import concourse.dve_ops as dve_ops
    from concourse.dve_spec import Spec, Src0, Src1
    from concourse.dve_uop import (
        ENABLE,
        AluInp,
        AluOp,
        DelayInp,
        InpSel,
        OutPath,
        OutSel,
        Trigger,
        UopConfig,
        UopDpConfig,
        DveOpSpec,
    )

    NAME = "FORGET_SCAN2_ANT"
    if NAME in dve_ops._SUB_OPCODE_FOR_NAME:
        return dve_ops.CUSTOM_DVE_SPECS[NAME + "_op"]

    ROW = 18  # rows 1..16 taken by production OPS; byte-36 field holds < 0x20
    assert ROW not in dve_ops._SUB_OPCODE_FOR_NAME.values()

    def _reference(in0, in1, s0, s1, imm2):
        # 2-interleaved chains: H[e] = f[e]*x[e] + (1 + s0*f[e]) * H[e-2]
        f = np.asarray(in0, np.float32)
        x = np.asarray(in1, np.float32)
        a = 1.0 + f * np.float32(s0)
        g = f * x
        out = np.empty_like(g)
        out[:, 0], out[:, 1] = g[:, 0], g[:, 1]
        for e in range(2, g.shape[1]):
            out[:, e] = g[:, e] + a[:, e] * out[:, e - 2]
        return out

    def _base_uop():
        u = UopConfig()
        # delay lanes: 0=f(SRC_0) 1=x(SRC_1) 2=CONST_0 3=ONE 4=a 5=g
        u.enable_input(InpSel.SRC_0, 1)
        u.enable_input(InpSel.SRC_1, 2)
        u.enable_input(InpSel.CONST_0, 3)
        u.enable_input(InpSel.ONE_F32, 4)
        u.require_inp0 = ENABLE
        u.require_inp1 = ENABLE
        dp = u.datapath_config
        dp[0].enable_alu(AluOp.MULTIPLY, AluInp.PREV_DELAY_0, AluInp.PREV_DELAY_2)
        dp[0].pass_through_delay(0, 1, 3)
        dp[1].enable_alu(AluOp.ADD, AluInp.PREV_DELAY_3, AluInp.PREV_ALU_OUT)
        dp[1].pass_through_delay(0, 1)
        dp[2].enable_alu(AluOp.MULTIPLY, AluInp.PREV_DELAY_0, AluInp.PREV_DELAY_1)
        dp[2].enable_delay_from_src(DelayInp.PREV_ALU_OUT, 4)  # a -> lane4
        dp[3].enable_alu(AluOp.MULTIPLY, AluInp.PREV_DELAY_4, AluInp.NEXT_ALU_OUT_A)
        dp[3].enable_delay_from_src(DelayInp.PREV_ALU_OUT, 5)  # g -> lane5
        dp[4].enable_alu(AluOp.ADD, AluInp.PREV_ALU_OUT, AluInp.PREV_DELAY_5)
        dp[4].alu_out_a_enable = ENABLE  # H feedback, read by B3 next cycle
        for k in (5, 6, 7):
            dp[k].pass_through_alu()
        u.enable_output(OutSel.ALU_OUT, OutPath.WR0_LO)
        return u

    def _build_uops(ver):
        assert ver == "v3", f"{NAME} authored for TRN2/v3 only ({ver})"
        seed = _base_uop()
        dp = seed.datapath_config
        dp[3] = UopDpConfig()
        dp[3].enable_alu(AluOp.BYPASS, AluInp.PREV_DELAY_4)
        dp[3].enable_delay_from_src(DelayInp.PREV_ALU_OUT, 5)
        dp[4] = UopDpConfig()
        dp[4].enable_alu(AluOp.BYPASS, AluInp.PREV_DELAY_5)  # H = g (state 0)
        dp[4].alu_out_a_enable = ENABLE
        seed.repeat_count = 2
        seed.trigger = (Trigger.SRC_TENSOR_DONE, Trigger.COUNT, Trigger.NONE)
        seed.next_uop = (0, 1, 0)
        steady = _base_uop()
        steady.trigger = (Trigger.SRC_TENSOR_DONE, Trigger.NONE, Trigger.NONE)
        steady.next_uop = (0, 0, 0)
        return [seed, steady]

    class _HandOp:
        """Duck-typed DveOp whose uOp program is hand-authored, not lowered."""

        name = NAME
        subdim = False
        spec = Spec(body=Src0 * Src1, reference=_reference)

        def __init__(self):
            self._cache = {}

        def compile(self, ver):
            if ver not in self._cache:
                s = DveOpSpec(
                    name=self.name, opcode=ROW, uops=_build_uops(ver), rd1_en=True
                )
                s.validate(ver)
                self._cache[ver] = s
            return self._cache[ver]

    op = _HandOp()
    dve_ops.OPS.append(op)
    dve_ops._SUB_OPCODE_FOR_NAME[NAME] = ROW
    dve_ops.CUSTOM_DVE_SPECS[NAME] = op.spec
    dve_ops.CUSTOM_DVE_SPECS[NAME + "_op"] = op  # stash for idempotent lookup
    return op


def _build():
    import concourse.bacc as bacc
    import concourse.mybir as mybir
    import concourse.tile as tile

    scan_op = _register_forget_scan()

    f16 = mybir.dt.float16
    u8 = mybir.dt.uint8
    i8 = mybir.dt.int8
    nc = bacc.Bacc("TRN2", target_bir_lowering=False, debug=False, num_devices=NCORES)
    f_in = nc.dram_tensor("f_in", [PB, NPAIR, T2], u8, kind="ExternalInput").ap()
    x_in = nc.dram_tensor("x_in", [PB, NPAIR, T2], i8, kind="ExternalInput").ap()
    h_out = nc.dram_tensor("h_out", [PB, NPAIR, T2], f16, kind="ExternalOutput").ap()

    H2 = T2 // 2
    with tile.TileContext(nc) as tc:
        with (
            tc.tile_pool(name="io", bufs=3) as io_pool,
            tc.tile_pool(name="hp", bufs=4) as h_pool,
            tc.tile_pool(name="hd", bufs=1) as hd_pool,
        ):
            deferred = {}
            blk0 = 0
            for ci, cb in enumerate(CHUNKS):
                bsl = slice(blk0, blk0 + cb)
                f_t = io_pool.tile([PB, cb, T2], u8, tag="f")
                nc.sync.dma_start(out=f_t[:], in_=f_in[:, bsl, :])
                x_t = io_pool.tile([PB, cb, T2], i8, tag="x")
                # early x loads ride the Scalar ring, idle until first stores
                x_eng = nc.scalar if ci < X_ON_SCALAR else nc.sync
                x_eng.dma_start(out=x_t[:], in_=x_in[:, bsl, :])
                if ci == len(CHUNKS) - 1:
                    # Sync ring is idle after the final load: flush deferred
                    # early stores there to fill the end-of-stream DMA gap
                    for dblk, dh in deferred.items():
                        nc.sync.dma_start(out=h_out[:, dblk, :], in_=dh[:])
                for j in range(cb):
                    blk = blk0 + j
                    if blk <= 1:
                        h_t = hd_pool.tile([PB, T2], f16, tag=f"hd{blk}", name=f"hd{blk}")
                    else:
                        h_t = h_pool.tile([PB, T2], f16, tag="h")
                    nc.vector._custom_dve(
                        scan_op,
                        out=h_t[:],
                        in0=f_t[:, j, :],
                        in1=x_t[:, j, :],
                        s0=-1.0 / F_SCALE,
                    )
                    if blk <= 1:
                        deferred[blk] = h_t
                    elif blk == NPAIR - 1:
                        # split the last store across both rings: halves the
                        # post-scan drain
                        nc.scalar.dma_start(out=h_out[:, blk, :H2], in_=h_t[:, :H2])
                        nc.sync.dma_start(out=h_out[:, blk, H2:], in_=h_t[:, H2:])
                    else:
                        nc.scalar.dma_start(out=h_out[:, blk, :], in_=h_t[:])
                blk0 += cb
    nc.compile()
    return nc


def _get_nc():
    if "nc" not in _cached:
        _cached["nc"] = _build()
    return _cached["nc"]


def _shard(arr):
    """[T, B, D] -> per-core [DS, NPAIR, 2T]: T reversed, partition-major,
    batch rows (2p, 2p+1) interleaved elementwise along the stream."""
    v = arr[::-1].transpose(2, 1, 0)  # [D, B, T] strided view, T reversed
    out = []
    for c in range(NCORES):
        s = v[DS * c : DS * (c + 1)]                  # [128, 16, 2048]
        s = s.reshape(DS, NPAIR, 2, T)                # [128, 8, 2, 2048]
        out.append(np.ascontiguousarray(s.transpose(0, 1, 3, 2)).reshape(DS, NPAIR, T2))
    return out


def _run(f, x, trace=False):
    from concourse.bass_utils import run_bass_kernel_spmd

    f = np.asarray(f, dtype=np.float32)
    x = np.asarray(x, dtype=np.float32)
    assert f.shape == (T, B, D) and x.shape == (T, B, D)

    # Quantize: f -> u8 (step 1/255), x -> i8 with global scale sx.
    fq = np.rint(f * np.float32(F_SCALE)).astype(np.uint8)
    sx = float(np.abs(x).max()) / 127.0
    sx = max(sx, 1e-30)
    xq = np.clip(np.rint(x * np.float32(1.0 / sx)), -127, 127).astype(np.int8)

    nc = _get_nc()
    f_shards = _shard(fq)
    x_shards = _shard(xq)
    in_maps = [{"f_in": f_shards[c], "x_in": x_shards[c]} for c in range(NCORES)]
    res = run_bass_kernel_spmd(nc, in_maps, core_ids=list(range(NCORES)), trace=trace)

    out = np.empty((T, B, D), dtype=np.float32)
    for c in range(NCORES):
        # H_c[d, p, 2k+j] -> out[t, 2p+j, DS*c + d] with k = T-1-t
        rr = res.results[c]["h_out"].reshape(DS, NPAIR, T, 2).transpose(2, 1, 3, 0)
        out[:, :, DS * c : DS * (c + 1)] = rr.reshape(T, B, DS)[::-1]
    out *= np.float32(sx / F_SCALE)
    return out.reshape(T * B, D), res


def kernel(f, x):
    return _run(f, x, trace=False)[0]


# revision 9
# speedup vs baseline: 1.1551x; 1.0434x over previous
"""Reverse-time forget-mult recurrence on 8 Trainium2 NeuronCores.

h_t = f_t*x_t + (1-f_t)*h_{t+1}, h_{T+1}=0, over [T=2048, B=16, D=1024].

Memory-bound problem, so bytes/element are minimized by quantization: f -> u8
(step 1/255), x -> i8 with one global scale sx = max|x|/127.  The recurrence
is a convex combination (|1-f|<1) so quantization error does not amplify; the
device scans in the integer-scaled domain H = (255/sx)*h with fp32 state and
fp16 output, and the host rescales by sx/255 (max rel err ~7e-3 vs the 2e-2
gate, validated in fp64 simulation).  An i8-output variant was measured
slower: the 1-byte write path throttles the scan below 1 element/cycle, which
costs more than the halved store traffic saves.

The whole per-element pipeline (dequant, gate product, and the sequential
scan) runs in ONE hand-authored custom DVE instruction at 1 element/cycle.
The stock tensor_tensor_scan costs 2 cycles/element: its state feedback
routes backward one pipeline stage (block k+1's a-flop read by block k),
which needs a one-cycle bubble between consecutive elements of one chain.
Instead, the host interleaves TWO independent (d,b)-lanes per stream:
element e reads the a-flop written one cycle earlier, which then belongs to
chain e-2 — its own chain — so the pipeline runs bubble-free at full rate.
Datapath (8 ALU blocks, elements stream through at 1/cycle, c = -1/255):

    B0: t = f * c                 B3: m = a * H_fb   (a-flop of B4)
    B1: a = 1 + t                 B4: H = m + g      (writes a-flop)
    B2: g = f * x                 B5-B7: pass-through -> fp16 write

A 2-element seed uOp starts each instruction (H = g, i.e. zero initial state
for both chains), then the steady uOp runs to end-of-stream.

Layout: D is sharded across the 8 cores (128 channels -> SBUF partitions).
Each core's [128, 16, 2048] shard is laid out T-reversed and pair-interleaved
as [128, 8, 4096]: pair p holds batch rows (2p, 2p+1) alternating per element.
f loads issue on the Sync HWDGE ring; early x loads ride the Scalar ring
(idle until the first stores).  One custom-DVE instruction per pair-block
[128, 4096]; fp16 stores on the Scalar ring, with the first two deferred to
the kernel tail on the then-idle Sync ring and the last one split across both
rings to shorten the drain.  Per-core HBM traffic: 4+4 MB in, 8 MB out.
Measured on TRN2: the 8 scans run back-to-back on Vector at 4.35 us each
(1.06 ns/element); ~7 us fixed NEFF/Tile preamble + ~2 us first-load latency
in front, ~4 us store drain + epilogue behind.
"""

import numpy as np

T, B, D = 2048, 16, 1024
NCORES = 8
DS = D // NCORES          # 128 channels per core -> the SBUF partition dim
PB = 128
NPAIR = B // 2            # 8 interleaved pair-blocks of [128, 2*T] per core
T2 = 2 * T
CHUNKS = (1, 1, 2, 2, 2)  # load-DMA granularity in pair-blocks
X_ON_SCALAR = 2           # chunks whose x-load rides the Scalar ring
F_SCALE = 255.0

_cached = {}


QN = T2 // 4              # sub-instruction stream length for split pair-blocks


def _register_forget_scan():
    """Register the hand-authored 2-chain interleaved linear-scan DVE ops.

    Three variants share one datapath:
      FORGET_SCAN2_ANT  — seed(2) -> steady: one whole pair-block, zero init.
      FSCAN2_HEAD_ANT   — seed(2) -> steady(QN-4) -> tail1(1) -> tail2(1):
                          first quarter of a split pair-block; the tail uOps
                          park chain A's state in B4's b-flop (elem QN-2) and
                          leave chain B's in the a-flop (elem QN-1).
      FSCAN2_CONT_ANT   — resume1(1) -> steady(QN-3) -> tail1 -> tail2:
                          continuation quarter; element 0 reads chain A's
                          state from the b-flop (NEXT_ALU_OUT_B), element 1
                          falls back to the a-flop read of the steady config.
    The a/b out-flops are not cleared between instructions, so consecutive
    sub-instructions of one pair-block hand the two chain states across.

    Returns dict {"single": op, "head": op, "cont": op}.
    """
    import concourse.dve_ops as dve_ops
    from concourse.dve_spec import Spec, Src0, Src1
    from concourse.dve_uop import (
        ENABLE,
        AluInp,
        AluOp,
        DelayInp,
        InpSel,
        OutPath,
        OutSel,
        Trigger,
        UopConfig,
        UopDpConfig,
        DveOpSpec,
    )

    KEY = "FORGET_SCAN2_OPS"
    if KEY in dve_ops.CUSTOM_DVE_SPECS:
        return dve_ops.CUSTOM_DVE_SPECS[KEY]

    def _reference(in0, in1, s0, s1, imm2):
        # 2-interleaved chains: H[e] = f[e]*x[e] + (1 + s0*f[e]) * H[e-2]
        # (reference ignores cross-instruction chaining; CoreSim-only)
        f = np.asarray(in0, np.float32)
        x = np.asarray(in1, np.float32)
        a = 1.0 + f * np.float32(s0)
        g = f * x
        out = np.empty_like(g)
        out[:, 0], out[:, 1] = g[:, 0], g[:, 1]
        for e in range(2, g.shape[1]):
            out[:, e] = g[:, e] + a[:, e] * out[:, e - 2]
        return out

    def _steady_uop():
        u = UopConfig()
        # delay lanes: 0=f(SRC_0) 1=x(SRC_1) 2=CONST_0 3=ONE 4=a 5=g
        u.enable_input(InpSel.SRC_0, 1)
        u.enable_input(InpSel.SRC_1, 2)
        u.enable_input(InpSel.CONST_0, 3)
        u.enable_input(InpSel.ONE_F32, 4)
        u.require_inp0 = ENABLE
        u.require_inp1 = ENABLE
        dp = u.datapath_config
        dp[0].enable_alu(AluOp.MULTIPLY, AluInp.PREV_DELAY_0, AluInp.PREV_DELAY_2)
        dp[0].pass_through_delay(0, 1, 3)
        dp[1].enable_alu(AluOp.ADD, AluInp.PREV_DELAY_3, AluInp.PREV_ALU_OUT)
        dp[1].pass_through_delay(0, 1)
        dp[2].enable_alu(AluOp.MULTIPLY, AluInp.PREV_DELAY_0, AluInp.PREV_DELAY_1)
        dp[2].enable_delay_from_src(DelayInp.PREV_ALU_OUT, 4)  # a -> lane4
        dp[3].enable_alu(AluOp.MULTIPLY, AluInp.PREV_DELAY_4, AluInp.NEXT_ALU_OUT_A)
        dp[3].enable_delay_from_src(DelayInp.PREV_ALU_OUT, 5)  # g -> lane5
        dp[4].enable_alu(AluOp.ADD, AluInp.PREV_ALU_OUT, AluInp.PREV_DELAY_5)
        dp[4].alu_out_a_enable = ENABLE  # H feedback, read by B3 next cycle
        for k in (5, 6, 7):
            dp[k].pass_through_alu()
        u.enable_output(OutSel.ALU_OUT, OutPath.WR0_LO)
        return u

    def _seed_uop():
        seed = _steady_uop()
        dp = seed.datapath_config
        dp[3] = UopDpConfig()
        dp[3].enable_alu(AluOp.BYPASS, AluInp.PREV_DELAY_4)
        dp[3].enable_delay_from_src(DelayInp.PREV_ALU_OUT, 5)
        dp[4] = UopDpConfig()
        dp[4].enable_alu(AluOp.BYPASS, AluInp.PREV_DELAY_5)  # H = g (state 0)
        dp[4].alu_out_a_enable = ENABLE
        return seed

    C, D_, N = Trigger.COUNT, Trigger.SRC_TENSOR_DONE, Trigger.NONE

    def _fsm(u, repeat, trig, nxt):
        u.repeat_count = repeat
        u.trigger = trig
        u.next_uop = nxt
        return u

    def _uops_single(ver):
        assert ver == "v3", f"forget-scan ops authored for TRN2/v3 only ({ver})"
        return [
            _fsm(_seed_uop(), 2, (D_, C, N), (0, 1, 0)),
            _fsm(_steady_uop(), 0, (D_, N, N), (0, 0, 0)),
        ]

    def _tail1_uop():
        t = _steady_uop()
        t.datapath_config[4].alu_out_b_enable = ENABLE  # chain A state -> b-flop
        return t

    def _resume1_uop():
        r = _steady_uop()
        # chain A's state comes from the previous sub-instruction's b-flop
        r.datapath_config[3].alu_src1 = AluInp.NEXT_ALU_OUT_B
        return r

    # repeat_cnt is an 8-bit field (max 255): cover the QN-4 = 1020 steady
    # elements with a chain of four 255-count steady uOps.
    assert QN - 4 == 4 * 255

    def _uops_head(ver):
        assert ver == "v3"
        us = [_fsm(_seed_uop(), 2, (D_, C, N), (0, 1, 0))]
        for i in range(4):
            us.append(_fsm(_steady_uop(), 255, (D_, C, N), (0, 2 + i, 0)))
        us.append(_fsm(_tail1_uop(), 1, (D_, C, N), (0, 6, 0)))
        us.append(_fsm(_steady_uop(), 0, (D_, N, N), (0, 0, 0)))  # tail2
        return us

    def _uops_cont(ver):
        assert ver == "v3"
        us = [
            _fsm(_resume1_uop(), 1, (D_, C, N), (0, 1, 0)),
            _fsm(_steady_uop(), 1, (D_, C, N), (0, 2, 0)),  # pad to QN-2 total
        ]
        for i in range(4):
            us.append(_fsm(_steady_uop(), 255, (D_, C, N), (0, 3 + i, 0)))
        us.append(_fsm(_tail1_uop(), 1, (D_, C, N), (0, 7, 0)))
        us.append(_fsm(_steady_uop(), 0, (D_, N, N), (0, 0, 0)))  # tail2
        return us

    class _HandOp:
        """Duck-typed DveOp whose uOp program is hand-authored, not lowered."""

        subdim = False

        def __init__(self, name, row, build):
            self.name, self._row, self._build = name, row, build
            self.spec = Spec(body=Src0 * Src1, reference=_reference)
            self._cache = {}

        def compile(self, ver):
            if ver not in self._cache:
                s = DveOpSpec(
                    name=self.name, opcode=self._row, uops=self._build(ver), rd1_en=True
                )
                s.validate(ver)
                self._cache[ver] = s
            return self._cache[ver]

    ops = {}
    for kind, name, row, build in (
        ("single", "FORGET_SCAN2_ANT", 18, _uops_single),
        ("head", "FSCAN2_HEAD_ANT", 19, _uops_head),
        ("cont", "FSCAN2_CONT_ANT", 20, _uops_cont),
    ):
        assert name not in dve_ops._SUB_OPCODE_FOR_NAME
        assert row not in dve_ops._SUB_OPCODE_FOR_NAME.values()
        op = _HandOp(name, row, build)
        dve_ops.OPS.append(op)
        dve_ops._SUB_OPCODE_FOR_NAME[name] = row
        dve_ops.CUSTOM_DVE_SPECS[name] = op.spec
        ops[kind] = op
    dve_ops.CUSTOM_DVE_SPECS[KEY] = ops
    return ops


def _build():
    import concourse.bacc as bacc
    import concourse.mybir as mybir
    import concourse.tile as tile

    scan_ops = _register_forget_scan()

    f16 = mybir.dt.float16
    u8 = mybir.dt.uint8
    i8 = mybir.dt.int8
    nc = bacc.Bacc("TRN2", target_bir_lowering=False, debug=False, num_devices=NCORES)
    f_in = nc.dram_tensor("f_in", [PB, NPAIR, T2], u8, kind="ExternalInput").ap()
    x_in = nc.dram_tensor("x_in", [PB, NPAIR, T2], i8, kind="ExternalInput").ap()
    h_out = nc.dram_tensor("h_out", [PB, NPAIR, T2], f16, kind="ExternalOutput").ap()

    H2 = T2 // 2
    with tile.TileContext(nc) as tc:
        with (
            tc.tile_pool(name="io", bufs=3) as io_pool,
            tc.tile_pool(name="hp", bufs=4) as h_pool,
            tc.tile_pool(name="hd", bufs=1) as hd_pool,
        ):
            def scan(op, h_ap, f_ap, x_ap):
                nc.vector._custom_dve(
                    op, out=h_ap, in0=f_ap, in1=x_ap, s0=-1.0 / F_SCALE
                )

            def scan_split(h_t, f_ap, x_ap, store_quarters=False):
                # four chained sub-instructions; chain state hands across via
                # the persistent a/b out-flops (HEAD parks, CONT resumes)
                for q in range(4):
                    qsl = slice(QN * q, QN * (q + 1))
                    scan(
                        scan_ops["head" if q == 0 else "cont"],
                        h_t[:, qsl], f_ap[:, qsl], x_ap[:, qsl],
                    )
                    if store_quarters:
                        nc.scalar.dma_start(
                            out=h_out[:, NPAIR - 1, qsl], in_=h_t[:, qsl]
                        )

            deferred = {}
            blk0 = 0
            for ci, cb in enumerate(CHUNKS):
                bsl = slice(blk0, blk0 + cb)
                f_t = io_pool.tile([PB, cb, T2], u8, tag="f")
                x_t = io_pool.tile([PB, cb, T2], i8, tag="x")
                if ci == 0:
                    # first chunk in halves on both rings so the first
                    # sub-scan starts as soon as possible
                    nc.sync.dma_start(out=f_t[:, :, :H2], in_=f_in[:, bsl, :H2])
                    nc.scalar.dma_start(out=x_t[:, :, :H2], in_=x_in[:, bsl, :H2])
                    nc.sync.dma_start(out=f_t[:, :, H2:], in_=f_in[:, bsl, H2:])
                    nc.scalar.dma_start(out=x_t[:, :, H2:], in_=x_in[:, bsl, H2:])
                else:
                    nc.sync.dma_start(out=f_t[:], in_=f_in[:, bsl, :])
                    # early x loads ride the Scalar ring (idle until stores)
                    x_eng = nc.scalar if ci < X_ON_SCALAR else nc.sync
                    x_eng.dma_start(out=x_t[:], in_=x_in[:, bsl, :])
                if ci == len(CHUNKS) - 1:
                    # Sync ring is idle after the final load: flush deferred
                    # early stores there to fill the end-of-stream DMA gap
                    for dblk, dh in deferred.items():
                        nc.sync.dma_start(out=h_out[:, dblk, :], in_=dh[:])
                for j in range(cb):
                    blk = blk0 + j
                    if blk <= 1:
                        h_t = hd_pool.tile([PB, T2], f16, tag=f"hd{blk}", name=f"hd{blk}")
                    else:
                        h_t = h_pool.tile([PB, T2], f16, tag="h")
                    if blk == 0:
                        scan_split(h_t, f_t[:, j, :], x_t[:, j, :])
                        deferred[blk] = h_t
                    elif blk == NPAIR - 1:
                        # quarter-granularity scans+stores shorten the drain
                        scan_split(h_t, f_t[:, j, :], x_t[:, j, :], store_quarters=True)
                    else:
                        scan(scan_ops["single"], h_t[:], f_t[:, j, :], x_t[:, j, :])
                        if blk <= 1:
                            deferred[blk] = h_t
                        else:
                            nc.scalar.dma_start(out=h_out[:, blk, :], in_=h_t[:])
                blk0 += cb
    nc.compile()
    return nc


def _get_nc():
    if "nc" not in _cached:
        _cached["nc"] = _build()
    return _cached["nc"]


def _shard(arr):
    """[T, B, D] -> per-core [DS, NPAIR, 2T]: T reversed, partition-major,
    batch rows (2p, 2p+1) interleaved elementwise along the stream."""
    v = arr[::-1].transpose(2, 1, 0)  # [D, B, T] strided view, T reversed
    out = []
    for c in range(NCORES):
        s = v[DS * c : DS * (c + 1)]                  # [128, 16, 2048]
        s = s.reshape(DS, NPAIR, 2, T)                # [128, 8, 2, 2048]
        out.append(np.ascontiguousarray(s.transpose(0, 1, 3, 2)).reshape(DS, NPAIR, T2))
    return out


def _run(f, x, trace=False):
    from concourse.bass_utils import run_bass_kernel_spmd

    f = np.asarray(f, dtype=np.float32)
    x = np.asarray(x, dtype=np.float32)
    assert f.shape == (T, B, D) and x.shape == (T, B, D)

    # Quantize: f -> u8 (step 1/255), x -> i8 with global scale sx.
    fq = np.rint(f * np.float32(F_SCALE)).astype(np.uint8)
    sx = float(np.abs(x).max()) / 127.0
    sx = max(sx, 1e-30)
    xq = np.clip(np.rint(x * np.float32(1.0 / sx)), -127, 127).astype(np.int8)

    nc = _get_nc()
    f_shards = _shard(fq)
    x_shards = _shard(xq)
    in_maps = [{"f_in": f_shards[c], "x_in": x_shards[c]} for c in range(NCORES)]
    res = run_bass_kernel_spmd(nc, in_maps, core_ids=list(range(NCORES)), trace=trace)

    out = np.empty((T, B, D), dtype=np.float32)
    for c in range(NCORES):
        # H_c[d, p, 2k+j] -> out[t, 2p+j, DS*c + d] with k = T-1-t
        rr = res.results[c]["h_out"].reshape(DS, NPAIR, T, 2).transpose(2, 1, 3, 0)
        out[:, :, DS * c : DS * (c + 1)] = rr.reshape(T, B, DS)[::-1]
    out *= np.float32(sx / F_SCALE)
    return out.reshape(T * B, D), res


def kernel(f, x):
    return _run(f, x, trace=False)[0]
